# revision 1
# baseline (speedup 1.0000x reference)
"""ChannelGate (topk_masking) Trainium2 Bass kernel.

Strategy: pure data parallel over batch (B=32 -> 4 samples per core x 8 cores).
Per core, per sample (x layout [C=512, HW=3136] as 4 c-tiles [128, 3136]):
  phase 1: stream x, compute channel stats (ACT copy+accum for sum, DVE
           reduce_max for max), pixel stats (PE f32r ones-matmul for sum,
           DVE tt-max combine + PE transpose + DVE psum reduce for max).
  phase 2: top-256 sorted extraction via DVE max8/match_replace on [8, 512]
           stat rows; tiny MLP on PE (interleave folded into host-split
           even/odd W1); 7x7 conv via DRAM padded buffer + im2col DMAs +
           PE f32r matmuls (BN folded into weights host-side).
  phase 3: re-stream x; gate = PE row broadcast + ACT sigmoid (per-partition
           channel scale) + one fused DVE/GPSIMD scalar_tensor_tensor
           out = (sig + 1) * x; DMA out.
"""
import os
import numpy as np
from contextlib import ExitStack

import concourse.bass as bass
import concourse.tile as tile
from concourse import bacc, mybir
from concourse import bass_utils

F32 = mybir.dt.float32
F32R = mybir.dt.float32r
F8 = mybir.dt.float8e4
BF16 = mybir.dt.bfloat16
AF = mybir.ActivationFunctionType
ALU = mybir.AluOpType
AX = mybir.AxisListType

B, C, H, W = 32, 512, 56, 56
HW = H * W            # 3136
S = 4                 # samples per core
NCORES = 8
G = 4                 # c-tiles of 128 per sample
RED = 32              # MLP hidden
NPIX_CH = 25          # ceil(3136/128) pixel chunks for transposes
CH512 = [(i * 512, min(512, HW - i * 512)) for i in range((HW + 511) // 512)]
PW = 62               # padded conv map width/height
NEG = -1.0e30


def r32(ap):
    return ap.bitcast(F32R)


def build_program():
    nc = bacc.Bacc("TRN2", target_bir_lowering=False, debug=False,
                   num_devices=NCORES)

    x_d = nc.dram_tensor("x", [S, C, HW], F32R, kind="ExternalInput")
    y_d = nc.dram_tensor("y", [S, C, HW], F32, kind="ExternalOutput")
    w1e_d = nc.dram_tensor("w1e", [64, 4 * RED], F32, kind="ExternalInput")
    w1o_d = nc.dram_tensor("w1o", [64, 4 * RED], F32, kind="ExternalInput")
    w2t_d = nc.dram_tensor("w2t", [RED, C], F32, kind="ExternalInput")
    b1_d = nc.dram_tensor("b1c", [RED, 1], F32, kind="ExternalInput")
    b2_d = nc.dram_tensor("b2c", [128, G], F32, kind="ExternalInput")
    wc_d = nc.dram_tensor("wc", [98, 1], F32, kind="ExternalInput")
    id_d = nc.dram_tensor("ident", [128, 128], F32, kind="ExternalInput")
    ssc_d = nc.dram_tensor("sortscale", [8, 1], F32, kind="ExternalInput")
    k2_d = nc.dram_tensor("k2c", [1, 1], F32, kind="ExternalInput")
    pad_d = nc.dram_tensor("pad0", [S * 2 * PW * PW], BF16, kind="ExternalInput")
    flat_d = nc.dram_tensor("flatscr", [S, NPIX_CH * 128], F32, kind="Internal")

    with tile.TileContext(nc) as tc:
        with ExitStack() as ctx:
            build_core(ctx, tc, x_d, y_d, w1e_d, w1o_d, w2t_d, b1_d, b2_d,
                       wc_d, id_d, ssc_d, k2_d, pad_d, flat_d)
    nc.compile()
    return nc


def build_core(ctx, tc, x_d, y_d, w1e_d, w1o_d, w2t_d, b1_d, b2_d, wc_d,
               id_d, ssc_d, k2_d, pad_d, flat_d):
    nc = tc.nc

    cpool = ctx.enter_context(tc.tile_pool(name="consts", bufs=1))
    xt_pool = ctx.enter_context(tc.tile_pool(name="xt", bufs=4))
    tmp_pool = ctx.enter_context(tc.tile_pool(name="tmp", bufs=2))
    scr_pool = ctx.enter_context(tc.tile_pool(name="scr", bufs=1))
    row_pool = ctx.enter_context(tc.tile_pool(name="rows", bufs=2))
    ss_pool = ctx.enter_context(tc.tile_pool(name="ss", bufs=3))
    ssl_pool = ctx.enter_context(tc.tile_pool(name="ssl", bufs=2))
    bc_pool = ctx.enter_context(tc.tile_pool(name="bc", bufs=1))
    sig_pool = ctx.enter_context(tc.tile_pool(name="sig", bufs=2))
    imt_pool = ctx.enter_context(tc.tile_pool(name="imt", bufs=1))

    ps_small = ctx.enter_context(tc.tile_pool(name="ps_small", bufs=2,
                                              space="PSUM"))
    ps_tr = ctx.enter_context(tc.tile_pool(name="ps_tr", bufs=2, space="PSUM"))
    ps_bc = ctx.enter_context(tc.tile_pool(name="ps_bc", bufs=2, space="PSUM"))

    # ---- constants / weights in SBUF ----
    ident = cpool.tile([128, 128], F32)
    nc.sync.dma_start(ident[:], id_d.ap())
    ones_col = cpool.tile([128, 1], F32)
    nc.vector.memset(ones_col[:], 1.0)
    ones_row = cpool.tile([1, 128], F32)
    nc.vector.memset(ones_row[:], 1.0)
    onesr_d = nc.dram_tensor("onesr", [128, 128], F32R, kind="ExternalInput")
    ones_r = cpool.tile([128, 128], F32R)
    nc.sync.dma_start(ones_r[:], onesr_d.ap())
    ident_bf = cpool.tile([128, 128], BF16)
    nc.vector.tensor_copy(ident_bf[:], ident[:])
    w1e = cpool.tile([64, 4 * RED], F32)
    nc.sync.dma_start(w1e[:], w1e_d.ap())
    w1o = cpool.tile([64, 4 * RED], F32)
    nc.sync.dma_start(w1o[:], w1o_d.ap())
    w2t = cpool.tile([RED, C], F32)
    nc.sync.dma_start(w2t[:], w2t_d.ap())
    b1 = cpool.tile([RED, 1], F32)
    nc.sync.dma_start(b1[:], b1_d.ap())
    b2 = cpool.tile([128, G], F32)
    nc.sync.dma_start(b2[:], b2_d.ap())
    wc = cpool.tile([98, 1], F32)
    nc.sync.dma_start(wc[:], wc_d.ap())
    wc_bf = cpool.tile([98, 1], BF16)
    nc.vector.tensor_copy(wc_bf[:], wc[:])
    sortscale = cpool.tile([8, 1], F32)
    nc.sync.dma_start(sortscale[:], ssc_d.ap())
    k2 = cpool.tile([1, 1], F32)
    nc.sync.dma_start(k2[:], k2_d.ap())

    sc_sum = [cpool.tile([128, 4], F32, tag=f"scs{g}", name=f"scs{g}") for g in range(G)]
    sc_max = [cpool.tile([128, 4], F32, tag=f"scm{g}", name=f"scm{g}") for g in range(G)]
    sc = [cpool.tile([128, 8], F32, tag=f"sc{g}", name=f"scq{g}") for g in range(G)]
    srt = cpool.tile([8, C], F32)                        # sort rows
    srtd = cpool.tile([8, 256], F32)                     # sorted top-256
    tq = [cpool.tile([64, 8], F32, tag=f"tq{q}", name=f"tq{q}") for q in range(4)]
    h_sb = cpool.tile([RED, S], F32)
    sqw = [cpool.tile([128, S], F32, tag=f"sqw{g}", name=f"sqw{g}") for g in range(G)]

    ssS = ss_pool.tile([S, HW], F32, tag="ssbig")        # pixel sums
    ssM = ss_pool.tile([S, HW], F32, tag="ssbig")        # pixel maxes

    # ================= PHASE 1: stats =================
    for s in range(S):
        xt = []
        for g in range(G):
            t = xt_pool.tile([128, HW], F32R, tag="t")
            nc.sync.dma_start(t[:], x_d.ap()[s, g * 128:(g + 1) * 128, :])
            xt.append(t)
            scr = scr_pool.tile([128, HW], F8)
            nc.scalar.activation(scr[:], t[:].bitcast(F32), AF.Copy,
                                 accum_out=sc_sum[g][:, s:s + 1])
            nc.vector.reduce_max(sc_max[g][:, s:s + 1], t[:].bitcast(F32),
                                 axis=AX.X)

        # pixel sums: ones.T @ x over all 4 c-tiles, f32r
        srow = row_pool.tile([1, HW], F32, tag="row")
        for (off, wdt) in CH512:
            ps = ps_bc.tile([1, 512], F32, tag='psb')
            for g in range(G):
                nc.tensor.matmul(ps[0:1, 0:wdt], ones_r[:, 0:1],
                                 xt[g][:, off:off + wdt],
                                 start=(g == 0), stop=(g == G - 1))
            nc.scalar.copy(srow[0:1, off:off + wdt], ps[0:1, 0:wdt])
        nc.sync.dma_start(ssS[s:s + 1, :], srow[:])

        # pixel maxes: combine 4 c-tiles (serial in-place chain, bf16 out)
        t1g = tmp_pool.tile([128, HW], BF16, tag="t1g")
        nc.vector.tensor_tensor(t1g[:], xt[0][:].bitcast(F32),
                                xt[1][:].bitcast(F32), op=ALU.max)
        mx = tmp_pool.tile([128, HW], BF16, tag="mx")
        nc.vector.tensor_tensor(mx[:], xt[2][:].bitcast(F32),
                                xt[3][:].bitcast(F32), op=ALU.max)
        nc.vector.tensor_tensor(mx[:], mx[:], t1g[:], op=ALU.max)
        # transpose 128-wide pixel chunks, reduce over c in psum
        ssl = ssl_pool.tile([128, NPIX_CH], F32)
        nc.vector.memset(ssl[:], 0.0)
        for j in range(NPIX_CH):
            wdt = min(128, HW - j * 128)
            pst = ps_tr.tile([128, 128], BF16, tag='pst')
            nc.tensor.transpose(pst[0:wdt, :], mx[:, j * 128:j * 128 + wdt],
                                ident_bf[:])
            nc.vector.reduce_max(ssl[0:wdt, j:j + 1], pst[0:wdt, :], axis=AX.X)
        # flatten [128, 25] -> DRAM pixel order via transpose + 2 DMAs
        psf = ps_tr.tile([NPIX_CH, 128], F32, tag='pst')
        nc.tensor.transpose(psf[:], ssl[:], ident[:])
        sslt = ssl_pool.tile([NPIX_CH, 128], F32)
        nc.scalar.copy(sslt[:], psf[:])
        nc.sync.dma_start(flat_d.ap()[s, :].rearrange("(p f) -> p f",
                                                      p=NPIX_CH), sslt[:])
        nc.sync.dma_start(ssM[s:s + 1, :],
                          flat_d.ap()[s, 0:HW].rearrange("(p f) -> p f", p=1))

    # ================= PHASE 2: topk sort + MLP =================
    for g in range(G):
        nc.sync.dma_start(sc[g][:, 0:4], sc_sum[g][:])
        nc.sync.dma_start(sc[g][:, 4:8], sc_max[g][:])
        pst = ps_small.tile([8, 128], F32, tag='pss')
        nc.tensor.transpose(pst[:], sc[g][:], ident[:])
        nc.scalar.activation(srt[:, g * 128:(g + 1) * 128], pst[:], AF.Copy,
                             scale=sortscale[:])
    for it in range(32):
        m8 = srtd[:, 8 * it:8 * it + 8]
        nc.vector.max(out=m8, in_=srt[:])
        nc.vector.match_replace(out=srt[:], in_to_replace=m8,
                                in_values=srt[:], imm_value=NEG)
    # transpose sorted rows into [64, 8] chunks (cols 0-3 t1, 4-7 t2)
    for q in range(4):
        pst = ps_small.tile([64, 8], F32, tag='pss')
        nc.tensor.transpose(pst[:], srtd[:, 64 * q:64 * q + 64],
                            ident[0:8, 0:8])
        nc.scalar.copy(tq[q][:], pst[:])
    # h = relu(W1e @ t1 + W1o @ t2 + b1)
    psh = ps_small.tile([RED, S], F32, tag='pss')
    for q in range(4):
        c0 = q * RED
        lhs_e = w1e[:, c0:c0 + RED]
        lhs_o = w1o[:, c0:c0 + RED]
        nc.tensor.matmul(psh[:], lhs_e, tq[q][:, 0:4],
                         start=(q == 0), stop=False)
        nc.tensor.matmul(psh[:], lhs_o, tq[q][:, 4:8],
                         start=False, stop=(q == 3))
    nc.scalar.activation(h_sb[:], psh[:], AF.Relu, bias=b1[:])
    # mlp_out per c-tile; squeeze_weight = relu(mlp_out + b2 + sigmoid(sc1*sc2))
    for g in range(G):
        psm = ps_small.tile([128, S], F32, tag='pss')
        nc.tensor.matmul(psm[:], w2t[:, g * 128:(g + 1) * 128], h_sb[:],
                         start=True, stop=True)
        prod = cpool.tile([128, S], F32, tag=f"prod{g}")
        nc.vector.tensor_tensor(prod[:], sc[g][:, 0:4], sc[g][:, 4:8],
                                op=ALU.mult)
        sigp = cpool.tile([128, S], F32, tag=f"sigp{g}")
        nc.scalar.activation(sigp[:], prod[:], AF.Sigmoid, scale=1.0 / HW)
        nc.vector.tensor_tensor(sigp[:], sigp[:], psm[:], op=ALU.add)
        nc.scalar.activation(sqw[g][:], sigp[:], AF.Relu, bias=b2[:, g:g + 1])

    # ================= SPATIAL PATH =================
    prodS = ss_pool.tile([S, HW], F32, tag="ssbig")
    nc.vector.tensor_tensor(prodS[:], ssS[:], ssM[:], op=ALU.mult)
    nc.scalar.activation(prodS[:], prodS[:], AF.Sigmoid, scale=1.0 / C)
    # conv: padded interiors -> im2col -> PE matmuls -> bn bias
    for s in range(S):
        for ci, src2 in ((0, ssS), (1, ssM)):
            base = ((s * 2 + ci) * PW + 3) * PW + 3
            dst = bass.AP(pad_d, base, [[PW, H], [1, W]])
            nc.gpsimd.dma_start(dst,
                                src2[s:s + 1, :].rearrange("p (h w) -> p h w",
                                                           h=H))
    cb = ss_pool.tile([S, HW], F32, tag="ssbig")
    for s in range(S):
        imt = imt_pool.tile([98, HW], BF16)
        for ci in range(2):
            for kh in range(7):
                base = ((s * 2 + ci) * PW + kh) * PW
                src = bass.AP(pad_d, base, [[1, 7], [PW, H], [1, W]])
                p0 = ci * 49 + kh * 7
                nc.sync.dma_start(imt[p0:p0 + 7, :], src)
        crow = row_pool.tile([1, HW], F32, tag="row")
        for (off, wdt) in CH512:
            psc = ps_bc.tile([1, 512], F32, tag='psb')
            nc.tensor.matmul(psc[0:1, 0:wdt], wc_bf[:],
                             imt[:, off:off + wdt], start=True, stop=True)
            nc.scalar.activation(crow[0:1, off:off + wdt], psc[0:1, 0:wdt],
                                 AF.Identity, bias=k2[0:1, 0:1])
        nc.sync.dma_start(cb[s:s + 1, :], crow[:])
    spw = ss_pool.tile([S, HW], F32, tag="ssbig")
    nc.vector.tensor_tensor(spw[:], cb[:], prodS[:], op=ALU.add)

    # ================= PHASE 3: gate =================
    for s in range(S):
        spr = row_pool.tile([1, HW], F32R, tag="rowr", bufs=1)
        nc.gpsimd.dma_start(spr[:], spw[s:s + 1, :])
        bcS = bc_pool.tile([128, HW], F32)
        for (off, wdt) in CH512:
            psb = ps_bc.tile([128, 512], F32, tag='psb')
            nc.tensor.matmul(psb[:, 0:wdt], ones_r[0:1, :],
                             spr[0:1, off:off + wdt],
                             start=True, stop=True)
            nc.scalar.copy(bcS[:, off:off + wdt], psb[:, 0:wdt])
        for g in range(G):
            xg = xt_pool.tile([128, HW], F32R, tag="t")
            nc.sync.dma_start(xg[:], x_d.ap()[s, g * 128:(g + 1) * 128, :])
            sg = sig_pool.tile([128, HW], F32)
            nc.scalar.activation(sg[:], bcS[:], AF.Sigmoid,
                                 scale=sqw[g][:, s:s + 1])
            nc.vector.scalar_tensor_tensor(sg[:], in0=sg[:], scalar=1.0,
                                           in1=xg[:].bitcast(F32),
                                           op0=ALU.add, op1=ALU.mult)
            nc.sync.dma_start(y_d.ap()[s, g * 128:(g + 1) * 128, :], sg[:])


_NC_CACHE = {}


def _get_program():
    if "nc" not in _NC_CACHE:
        _NC_CACHE["nc"] = build_program()
    return _NC_CACHE["nc"]


def _host_params(w1, b1, w2, b2, conv_w, bn_gamma, bn_beta, bn_mean, bn_var):
    w1 = np.asarray(w1, np.float32)
    w2 = np.asarray(w2, np.float32)
    b1 = np.asarray(b1, np.float32)
    b2 = np.asarray(b2, np.float32)
    conv_w = np.asarray(conv_w, np.float32)

    w1e = np.ascontiguousarray(
        w1[:, 0::2].T.reshape(4, 64, RED).transpose(1, 0, 2).reshape(64, 4 * RED))
    w1o = np.ascontiguousarray(
        w1[:, 1::2].T.reshape(4, 64, RED).transpose(1, 0, 2).reshape(64, 4 * RED))
    w2t = np.ascontiguousarray(w2.T)                    # [32, 512]
    b1c = b1.reshape(RED, 1).copy()
    b2c = np.ascontiguousarray(b2.reshape(G, 128).T)    # [128, G]

    bn_scale = float(bn_gamma[0]) / np.sqrt(float(bn_var[0]) + 1e-5)
    k2 = float(bn_beta[0]) - float(bn_mean[0]) * bn_scale
    wcf = conv_w[0].astype(np.float64) * bn_scale       # [2, 7, 7]
    wcf = wcf.copy()
    wcf[0] /= C                                         # mean channel fold
    wc = wcf.reshape(98, 1).astype(np.float32)

    sortscale = np.concatenate([np.full(4, 1.0 / HW, np.float32),
                                np.ones(4, np.float32)]).reshape(8, 1)
    ident = np.eye(128, dtype=np.float32)
    k2c = np.array([[k2]], np.float32)
    onesr = np.ones((128, 128), np.float32)
    import ml_dtypes
    pad0 = np.zeros(S * 2 * PW * PW, ml_dtypes.bfloat16)
    return dict(w1e=w1e, w1o=w1o, w2t=w2t, b1c=b1c, b2c=b2c, wc=wc,
                ident=ident, sortscale=sortscale, k2c=k2c, pad0=pad0,
                onesr=onesr)


def kernel(x, w1, b1, w2, b2, conv_w, bn_gamma, bn_beta, bn_mean, bn_var):
    x = np.asarray(x, np.float32)
    params = _host_params(w1, b1, w2, b2, conv_w,
                          bn_gamma, bn_beta, bn_mean, bn_var)
    nc = _get_program()

    xr = x.reshape(B, C, HW)
    in_maps = []
    for k in range(NCORES):
        m = {"x": np.ascontiguousarray(xr[k * S:(k + 1) * S])}
        m.update(params)
        in_maps.append(m)

    res = bass_utils.run_bass_kernel_spmd(nc, in_maps,
                                          core_ids=list(range(NCORES)))
    out = np.concatenate([res.results[k]["y"] for k in range(NCORES)], axis=0)
    return out.reshape(B, C, H, W).astype(np.float32)



# revision 9
# speedup vs baseline: 1.3210x; 1.3210x over previous
"""ChannelGate (topk_masking) Trainium2 Bass kernel.

Strategy: pure data parallel over batch (B=32 -> 4 samples per core x 8 cores).
v2: single-pass over x. Each core loads its 4 samples' x once as resident
bf16 tiles (SWDGE cast-DMA f32->bf16), computes all stats from SBUF,
and gates in place -- no second HBM read. y is written bf16 and upcast
on host (tolerance 2e-2 >> bf16 noise).

Per sample (x layout [C=512, HW=3136] as 4 c-tiles [128, 3136] bf16):
  stats:  channel sum+max via one fused DVE tensor_tensor_reduce each
          (halves combined with add/max, accum reduces the rest);
          pixel sum via PE ones-matmul; pixel max via 3 in-place DVE
          TT-max + PE transposes + fused TTR reduces.
  topk:   channel maxes epsilon-perturbed by channel index to break
          bf16 ties, then top-256 sorted extraction via DVE max8/
          match_replace on [8, 512]; tiny MLP on PE.
  spatial: 7x7 conv via DRAM padded buffer + im2col; conv weights
          replicated to 128 columns so one PE matmul yields the
          BROADCAST spatial weight (sigmoid term + BN bias fused in as
          extra im2col rows 98/99).
  gate:   ACT sigmoid (per-partition channel scale) + DVE
          scalar_tensor_tensor out = (sig + 1) * x; DMA out bf16.
"""
import numpy as np
from contextlib import ExitStack

import concourse.bass as bass
import concourse.tile as tile
from concourse import bacc, mybir
from concourse import bass_utils
from concourse.bass_isa import ReduceOp

F32 = mybir.dt.float32
F32R = mybir.dt.float32r
BF16 = mybir.dt.bfloat16
AF = mybir.ActivationFunctionType
ALU = mybir.AluOpType
AX = mybir.AxisListType

B, C, H, W = 32, 512, 56, 56
HW = H * W            # 3136
S = 4                 # samples per core
NCORES = 8
G = 4                 # c-tiles of 128 per sample
RED = 32              # MLP hidden
NPIX_CH = 25          # ceil(3136/128) pixel chunks for transposes
CH512 = [(i * 512, min(512, HW - i * 512)) for i in range((HW + 511) // 512)]
PW = 62               # padded conv map width/height
NEG = -1.0e30
EPS_TIE = 2.0e-5      # channel-index tie-break for bf16 channel maxes


def build_program():
    nc = bacc.Bacc("TRN2", target_bir_lowering=False, debug=False,
                   num_devices=NCORES)

    x_d = nc.dram_tensor("x", [S, C, HW], F32, kind="ExternalInput")
    y_d = nc.dram_tensor("y", [S, C, HW], BF16, kind="ExternalOutput")
    w1e_d = nc.dram_tensor("w1e", [64, 4 * RED], F32, kind="ExternalInput")
    w1o_d = nc.dram_tensor("w1o", [64, 4 * RED], F32, kind="ExternalInput")
    w2t_d = nc.dram_tensor("w2t", [RED, C], F32, kind="ExternalInput")
    b1_d = nc.dram_tensor("b1c", [RED, 1], F32, kind="ExternalInput")
    b2_d = nc.dram_tensor("b2c", [128, G], F32, kind="ExternalInput")
    wcr_d = nc.dram_tensor("wcrep", [100, 128], BF16, kind="ExternalInput")
    id_d = nc.dram_tensor("ident", [128, 128], F32, kind="ExternalInput")
    ssc_d = nc.dram_tensor("sortscale", [8, 1], F32, kind="ExternalInput")
    cid_d = nc.dram_tensor("cidxeps", [128, G], F32, kind="ExternalInput")
    or_d = nc.dram_tensor("onesrow", [1, HW], BF16, kind="ExternalInput")
    pad_d = nc.dram_tensor("pad0", [S * 2 * PW * PW], BF16, kind="ExternalInput")

    with tile.TileContext(nc) as tc:
        with ExitStack() as ctx:
            build_core(ctx, tc, x_d, y_d, w1e_d, w1o_d, w2t_d, b1_d, b2_d,
                       wcr_d, id_d, ssc_d, cid_d, or_d, pad_d)
    nc.compile()
    return nc


def build_core(ctx, tc, x_d, y_d, w1e_d, w1o_d, w2t_d, b1_d, b2_d, wcr_d,
               id_d, ssc_d, cid_d, or_d, pad_d):
    nc = tc.nc

    cpool = ctx.enter_context(tc.tile_pool(name="consts", bufs=1))
    xb_pool = ctx.enter_context(tc.tile_pool(name="xb", bufs=1))
    scr_pool = ctx.enter_context(tc.tile_pool(name="scr", bufs=2))
    mx_pool = ctx.enter_context(tc.tile_pool(name="mx", bufs=2))
    row_pool = ctx.enter_context(tc.tile_pool(name="rows", bufs=1))
    ss_pool = ctx.enter_context(tc.tile_pool(name="ss", bufs=1))
    bc_pool = ctx.enter_context(tc.tile_pool(name="bc", bufs=2))
    sig_pool = ctx.enter_context(tc.tile_pool(name="sig", bufs=2))
    imt_pool = ctx.enter_context(tc.tile_pool(name="imt", bufs=2))

    ps_row = ctx.enter_context(tc.tile_pool(name="ps_row", bufs=2,
                                            space="PSUM"))
    ps_small = ctx.enter_context(tc.tile_pool(name="ps_small", bufs=2,
                                              space="PSUM"))
    ps_bc = ctx.enter_context(tc.tile_pool(name="ps_bc", bufs=2, space="PSUM"))

    # ---- constants / weights in SBUF ----
    ident = cpool.tile([128, 128], F32)
    nc.sync.dma_start(ident[:], id_d.ap())
    ones_bf = cpool.tile([128, 1], BF16)
    nc.vector.memset(ones_bf[:], 1.0)
    w1e = cpool.tile([64, 4 * RED], F32)
    nc.sync.dma_start(w1e[:], w1e_d.ap())
    w1o = cpool.tile([64, 4 * RED], F32)
    nc.sync.dma_start(w1o[:], w1o_d.ap())
    w2t = cpool.tile([RED, C], F32)
    nc.sync.dma_start(w2t[:], w2t_d.ap())
    b1 = cpool.tile([RED, 1], F32)
    nc.sync.dma_start(b1[:], b1_d.ap())
    b2 = cpool.tile([128, G], F32)
    nc.sync.dma_start(b2[:], b2_d.ap())
    wc_rep = cpool.tile([100, 128], BF16)
    nc.sync.dma_start(wc_rep[:], wcr_d.ap())
    sortscale = cpool.tile([8, 1], F32)
    nc.sync.dma_start(sortscale[:], ssc_d.ap())
    cidx = cpool.tile([128, G], F32)
    nc.sync.dma_start(cidx[:], cid_d.ap())

    sc = [cpool.tile([128, 8], F32, tag=f"sc{g}", name=f"scq{g}") for g in range(G)]
    srt = cpool.tile([8, C], F32)                        # sort rows
    srtd = cpool.tile([8, 256], F32)                     # sorted top-256
    tq = [cpool.tile([64, 8], F32, tag=f"tq{q}", name=f"tq{q}") for q in range(4)]
    h_sb = cpool.tile([RED, S], F32)
    sqw = [cpool.tile([128, S], F32, tag=f"sqw{g}", name=f"sqw{g}") for g in range(G)]

    ssS = ss_pool.tile([S, HW], BF16, tag="ssS")         # pixel sums (raw)
    ssM = ss_pool.tile([S, HW], BF16, tag="ssM")         # pixel maxes

    # resident x tiles (bf16)
    xb = [[xb_pool.tile([128, HW], BF16, tag=f"xb{s}_{g}", name=f"xb{s}_{g}")
           for g in range(G)] for s in range(S)]

    # ================= PHASE 1: load + stats =================
    # all loads first: keeps the in-order Pool (SWDGE) queue free of
    # compute ops so DMA issue is never stalled behind partition reduces
    for s in range(S):
        for g in range(G):
            # SWDGE cast-DMA: f32 HBM -> bf16 SBUF
            nc.gpsimd.dma_start(xb[s][g][:],
                                x_d.ap()[s, g * 128:(g + 1) * 128, :])
    for s in range(S):
        for g in range(G):
            t = xb[s][g]
            # channel sum: halves added, accum-reduce the rest (one DVE op)
            scrA = scr_pool.tile([128, HW // 2], BF16, tag="scrA")
            nc.vector.scalar_tensor_tensor(
                scrA[:], in0=t[:, 0:HW // 2], scalar=1.0, in1=t[:, HW // 2:HW],
                op0=ALU.mult, op1=ALU.add, accum_out=sc[g][:, s:s + 1])
            # channel max: bf16 TT tree + short reduce
            scrB = scr_pool.tile([128, HW // 2], BF16, tag="scrB")
            nc.vector.tensor_tensor(scrB[:], t[:, 0:HW // 2], t[:, HW // 2:HW],
                                    op=ALU.max)
            scrC = scr_pool.tile([128, HW // 4], BF16, tag="scrC")
            nc.vector.tensor_tensor(scrC[:], scrB[:, 0:HW // 4],
                                    scrB[:, HW // 4:HW // 2], op=ALU.max)
            nc.vector.reduce_max(sc[g][:, 4 + s:5 + s], scrC[:], axis=AX.X)

        # pixel sums: ones.T @ x over all 4 c-tiles (bf16 PE)
        srow = row_pool.tile([1, HW], BF16, tag="row")
        for (off, wdt) in CH512:
            ps = ps_row.tile([1, 512], F32, tag='psr')
            for g in range(G):
                nc.tensor.matmul(ps[0:1, 0:wdt], ones_bf[:],
                                 xb[s][g][:, off:off + wdt],
                                 start=(g == 0), stop=(g == G - 1))
            nc.scalar.activation(srow[0:1, off:off + wdt], ps[0:1, 0:wdt],
                                 AF.Copy)
        nc.sync.dma_start(ssS[s:s + 1, :], srow[:])

        # pixel maxes: combine 4 c-tiles in place, then Q7 partition reduce
        mx = mx_pool.tile([128, HW], BF16, tag="mx")
        nc.vector.tensor_tensor(mx[:], xb[s][0][:], xb[s][1][:], op=ALU.max)
        nc.vector.tensor_tensor(mx[:], mx[:], xb[s][2][:], op=ALU.max)
        nc.vector.tensor_tensor(mx[:], mx[:], xb[s][3][:], op=ALU.max)
        nc.gpsimd.partition_all_reduce(mx[:], mx[:], 128, ReduceOp.max)
        nc.sync.dma_start(ssM[s:s + 1, :], mx[0:1, :])

    # ================= PHASE 2: topk sort + MLP =================
    for g in range(G):
        # epsilon tie-break on channel maxes (bf16 values collide)
        nc.vector.tensor_scalar(out=sc[g][:, 4:8], in0=sc[g][:, 4:8],
                                scalar1=cidx[:, g:g + 1], scalar2=None,
                                op0=ALU.add)
        pst = ps_small.tile([8, 128], F32, tag='pss')
        nc.tensor.transpose(pst[:], sc[g][:], ident[:])
        nc.scalar.activation(srt[:, g * 128:(g + 1) * 128], pst[:], AF.Copy,
                             scale=sortscale[:])
    for it in range(32):
        m8 = srtd[:, 8 * it:8 * it + 8]
        nc.vector.max(out=m8, in_=srt[:])
        nc.vector.match_replace(out=srt[:], in_to_replace=m8,
                                in_values=srt[:], imm_value=NEG)
    # transpose sorted rows into [64, 8] chunks (cols 0-3 t1, 4-7 t2)
    for q in range(4):
        pst = ps_small.tile([64, 8], F32, tag='pss')
        nc.tensor.transpose(pst[:], srtd[:, 64 * q:64 * q + 64],
                            ident[0:8, 0:8])
        nc.scalar.copy(tq[q][:], pst[:])
    # h = relu(W1e @ t1 + W1o @ t2 + b1)
    psh = ps_small.tile([RED, S], F32, tag='pss')
    for q in range(4):
        c0 = q * RED
        nc.tensor.matmul(psh[:], w1e[:, c0:c0 + RED], tq[q][:, 0:4],
                         start=(q == 0), stop=False)
        nc.tensor.matmul(psh[:], w1o[:, c0:c0 + RED], tq[q][:, 4:8],
                         start=False, stop=(q == 3))
    nc.scalar.activation(h_sb[:], psh[:], AF.Relu, bias=b1[:])
    # mlp_out per c-tile; squeeze_weight = relu(mlp_out + b2 + sigmoid(sc1*sc2))
    for g in range(G):
        psm = ps_small.tile([128, S], F32, tag='pss')
        nc.tensor.matmul(psm[:], w2t[:, g * 128:(g + 1) * 128], h_sb[:],
                         start=True, stop=True)
        prod = cpool.tile([128, S], F32, tag=f"prod{g}")
        nc.vector.tensor_tensor(prod[:], sc[g][:, 0:4], sc[g][:, 4:8],
                                op=ALU.mult)
        sigp = cpool.tile([128, S], F32, tag=f"sigp{g}")
        nc.scalar.activation(sigp[:], prod[:], AF.Sigmoid, scale=1.0 / HW)
        nc.vector.tensor_tensor(sigp[:], sigp[:], psm[:], op=ALU.add)
        nc.scalar.activation(sqw[g][:], sigp[:], AF.Relu, bias=b2[:, g:g + 1])

    # ================= SPATIAL PATH =================
    prodS = ss_pool.tile([S, HW], BF16, tag="prodS")
    nc.vector.tensor_tensor(prodS[:], ssS[:], ssM[:], op=ALU.mult)
    nc.scalar.activation(prodS[:], prodS[:], AF.Sigmoid, scale=1.0 / C)
    # conv: padded interiors -> im2col -> fused conv+sig+bias+broadcast
    for s in range(S):
        for ci, src2 in ((0, ssS), (1, ssM)):
            base = ((s * 2 + ci) * PW + 3) * PW + 3
            dst = bass.AP(pad_d, base, [[PW, H], [1, W]])
            nc.gpsimd.dma_start(dst,
                                src2[s:s + 1, :].rearrange("p (h w) -> p h w",
                                                           h=H))
    for s in range(S):
        imt = imt_pool.tile([100, HW], BF16)
        for ci in range(2):
            for kh in range(7):
                base = ((s * 2 + ci) * PW + kh) * PW
                src = bass.AP(pad_d, base, [[1, 7], [PW, H], [1, W]])
                p0 = ci * 49 + kh * 7
                nc.sync.dma_start(imt[p0:p0 + 7, :], src)
        # row 98: sigmoid(ss1*ss2) weight 1.0; row 99: ones, weight k2 (BN)
        nc.sync.dma_start(imt[98:99, :], prodS[s:s + 1, :])
        nc.sync.dma_start(imt[99:100, :], or_d.ap())
        # fused conv + broadcast: wc_rep [100,128] makes psum [128,wdt] be
        # the spatial weight replicated across all 128 partitions
        bcS = bc_pool.tile([128, HW], BF16, tag="bcS", name=f"bcS{s}")
        for (off, wdt) in CH512:
            psb = ps_bc.tile([128, 512], F32, tag='psb')
            nc.tensor.matmul(psb[:, 0:wdt], wc_rep[:],
                             imt[:, off:off + wdt], start=True, stop=True)
            nc.scalar.activation(bcS[:, off:off + wdt], psb[:, 0:wdt],
                                 AF.Copy)

        # ================= PHASE 3: gate =================
        for g in range(G):
            sg = sig_pool.tile([128, HW], BF16, tag="sg")
            nc.scalar.activation(sg[:], bcS[:], AF.Sigmoid,
                                 scale=sqw[g][:, s:s + 1])
            nc.vector.scalar_tensor_tensor(sg[:], in0=sg[:], scalar=1.0,
                                           in1=xb[s][g][:],
                                           op0=ALU.add, op1=ALU.mult)
            nc.sync.dma_start(y_d.ap()[s, g * 128:(g + 1) * 128, :], sg[:])


_NC_CACHE = {}


def _get_program():
    if "nc" not in _NC_CACHE:
        _NC_CACHE["nc"] = build_program()
    return _NC_CACHE["nc"]


def _host_params(w1, b1, w2, b2, conv_w, bn_gamma, bn_beta, bn_mean, bn_var):
    import ml_dtypes
    w1 = np.asarray(w1, np.float32)
    w2 = np.asarray(w2, np.float32)
    b1 = np.asarray(b1, np.float32)
    b2 = np.asarray(b2, np.float32)
    conv_w = np.asarray(conv_w, np.float32)

    w1e = np.ascontiguousarray(
        w1[:, 0::2].T.reshape(4, 64, RED).transpose(1, 0, 2).reshape(64, 4 * RED))
    w1o = np.ascontiguousarray(
        w1[:, 1::2].T.reshape(4, 64, RED).transpose(1, 0, 2).reshape(64, 4 * RED))
    w2t = np.ascontiguousarray(w2.T)                    # [32, 512]
    b1c = b1.reshape(RED, 1).copy()
    b2c = np.ascontiguousarray(b2.reshape(G, 128).T)    # [128, G]

    bn_scale = float(bn_gamma[0]) / np.sqrt(float(bn_var[0]) + 1e-5)
    k2 = float(bn_beta[0]) - float(bn_mean[0]) * bn_scale
    wcf = conv_w[0].astype(np.float64) * bn_scale       # [2, 7, 7]
    wcf = wcf.copy()
    wcf[0] /= C                                         # mean channel fold
    # conv taps + sigmoid row (1.0) + BN-bias row (k2), replicated 128 wide
    wc100 = np.concatenate([wcf.reshape(98), [1.0], [k2]]).astype(np.float32)
    wcrep = np.repeat(wc100[:, None], 128, axis=1).astype(ml_dtypes.bfloat16)

    sortscale = np.concatenate([np.full(4, 1.0 / HW, np.float32),
                                np.ones(4, np.float32)]).reshape(8, 1)
    ident = np.eye(128, dtype=np.float32)
    cidxeps = (EPS_TIE * (np.arange(128)[:, None]
                          + 128.0 * np.arange(G)[None, :])).astype(np.float32)
    onesrow = np.ones((1, HW), ml_dtypes.bfloat16)
    pad0 = np.zeros(S * 2 * PW * PW, ml_dtypes.bfloat16)
    return dict(w1e=w1e, w1o=w1o, w2t=w2t, b1c=b1c, b2c=b2c, wcrep=wcrep,
                ident=ident, sortscale=sortscale, cidxeps=cidxeps,
                onesrow=onesrow, pad0=pad0)


def kernel(x, w1, b1, w2, b2, conv_w, bn_gamma, bn_beta, bn_mean, bn_var):
    x = np.asarray(x, np.float32)
    params = _host_params(w1, b1, w2, b2, conv_w,
                          bn_gamma, bn_beta, bn_mean, bn_var)
    nc = _get_program()

    xr = x.reshape(B, C, HW)
    in_maps = []
    for k in range(NCORES):
        m = {"x": np.ascontiguousarray(xr[k * S:(k + 1) * S])}
        m.update(params)
        in_maps.append(m)

    res = bass_utils.run_bass_kernel_spmd(nc, in_maps,
                                          core_ids=list(range(NCORES)))
    out = np.concatenate([np.asarray(res.results[k]["y"])
                          for k in range(NCORES)], axis=0)
    return out.reshape(B, C, H, W).astype(np.float32)
